# revision 1
# baseline (speedup 1.0000x reference)
"""Trainium2 Bass kernel for voxel-CNN + point-MLP (nn_CNN_Baseline_62646392980178).

Sharding: data-parallel over batch B=8 across 8 NeuronCores (one sample per
core); params replicated.  Host prep (per sample): pack each voxel's full
3x3x3x4ci input neighborhood into a 256-byte bf16 row (x_nbr [262144, 128]),
sort points by voxel, build int16 gather-index tables.  Device: dma_gather
rows at point voxels -> PE-transpose into im2col [108, P] -> conv matmul ->
MLP (128/128/256/10) on TensorE.  BatchNorm is training-mode over (B, P), so
per-channel sums are allreduced across cores; the global-max feature's
layer-1 contribution is a per-core per-channel constant and is folded into
the BN statistics (cross terms in the allreduce payload) and the layer-1
bias, which removes the max-pool barrier from the matmul pipeline.
"""

import sys

sys.path.insert(0, "/opt/trn_rl_repo")

import numpy as np
import ml_dtypes

import concourse.bass as bass
import concourse.bacc as bacc
import concourse.mybir as mybir
import concourse.tile as tile
from concourse.bass_utils import run_bass_kernel_spmd

BF16 = ml_dtypes.bfloat16
F32 = mybir.dt.float32
F32R = mybir.dt.float32r
BF = mybir.dt.bfloat16
I16 = mybir.dt.int16
AF = mybir.ActivationFunctionType
OP = mybir.AluOpType

GRID = 64
EPS = 1e-5
B = 8
P = 16384
CIN = 4
COUT = 32
K_IM = 108
ROW = 128           # bf16 elems per packed row (256B)
NBLK = 16
BLK = P // NBLK     # 1024
NCH = P // 512      # 32
D1, D2, D3, DO = 128, 128, 256, 10
N_TOT = float(B * P)

_prog_cache = [None]


def _pack_host(x, pt_loc):
    idx = np.clip(np.floor(pt_loc).astype(np.int64), 0, GRID - 1)
    lin = (idx[..., 0] * GRID + idx[..., 1]) * GRID + idx[..., 2]
    order = np.argsort(lin, axis=1, kind="stable")
    lin_s = np.take_along_axis(lin, order, axis=1)
    xp = np.pad(x, ((0, 0), (0, 0), (1, 1), (1, 1), (1, 1)))
    sw = np.lib.stride_tricks.sliding_window_view(xp, (3, 3, 3), axis=(2, 3, 4))
    nbr = np.ascontiguousarray(sw.transpose(0, 2, 3, 4, 1, 5, 6, 7)).reshape(
        B, GRID ** 3, K_IM)
    x_nbr = np.zeros((B, GRID ** 3, ROW), dtype=BF16)
    x_nbr[:, :, :K_IM] = nbr.astype(BF16)
    return x_nbr, lin_s, order


def _build_idx_tables(lin_s):
    bases, tiles = [], []
    for k in range(NBLK):
        seg = lin_s[k * BLK:(k + 1) * BLK]
        base = int(seg[0])
        assert int(seg[-1]) - base < 32768, "gather block span exceeds int16"
        rel = (seg - base).astype(np.int16)
        wrapped = rel.reshape(BLK // 16, 16).T          # [16, BLK//16]
        t = np.tile(wrapped, (8, 1))                    # replicate per Q7 core
        bases.append(base)
        tiles.append(t)
    return bases, np.stack(tiles, axis=1)               # [128, NBLK, BLK//16]


def _build_program():
    nc = bacc.Bacc("TRN2", target_bir_lowering=False, debug=False, num_devices=8)

    def din(name, shape, dt):
        return nc.dram_tensor(name, shape, dt, kind="ExternalInput").ap()

    srcs = [din(f"src{k}", [32768, ROW], BF) for k in range(NBLK)]
    idxs = din("idx", [128, NBLK, BLK // 16], I16)
    ptT = din("ptT", [6, P], BF)
    ident = din("ident", [128, 128], BF)
    wext = din("wext", [K_IM, COUT], BF)
    b3dp = din("b3d", [COUT, 1], F32)
    w1pt = din("w1pt", [6, D1], BF)
    w1cv = din("w1cv", [COUT, D1], BF)
    w1gl = din("w1gl", [COUT, D1], BF)
    w2p = din("w2", [D1, D2], BF)
    w3ap = din("w3a", [D2, 128], BF)
    w3bp = din("w3b", [D2, 128], BF)
    woap = din("wo_a", [128, DO], BF)
    wobp = din("wo_b", [128, DO], BF)
    gb1p = din("gb1", [D1, 2], F32)
    gb2p = din("gb2", [D2, 2], F32)
    gb3ap = din("gb3a", [128, 2], F32)
    gb3bp = din("gb3b", [128, 2], F32)
    boutp = din("bout", [128, 8, DO], F32)
    out = nc.dram_tensor("out", [128, P // 128, DO], F32, kind="ExternalOutput").ap()

    import os as _os
    _stage = _os.environ.get("K_STAGE", "full")
    _sub = _os.environ.get("K_SUB", "l1")
    with tile.TileContext(nc) as tc:
        with tc.tile_pool(name="sb", bufs=1) as sb, \
             tc.tile_pool(name="ps", bufs=2, space="PSUM") as ps, \
             tc.tile_pool(name="psy", bufs=1, space="PSUM") as psy, \
             tc.tile_pool(name="imp", bufs=3) as imp, \
             tc.tile_pool(name="dramp", bufs=1, space="DRAM") as dramp:

            idx_sb = sb.tile([128, NBLK, BLK // 16], I16, tag="idx")
            nc.sync.dma_start(out=idx_sb[:], in_=idxs[:])

            id_sb = sb.tile([128, 128], BF, tag="ident")
            nc.sync.dma_start(out=id_sb[:], in_=ident[:])

            def loadw(ap_, shape, dt, tag):
                t = sb.tile(shape, dt, tag=tag)
                nc.sync.dma_start(out=t[:], in_=ap_[:])
                return t

            wext_sb = loadw(wext, [K_IM, COUT], BF, "wext")
            b3d_sb = loadw(b3dp, [COUT, 1], F32, "b3d")
            w1pt_sb = loadw(w1pt, [6, D1], BF, "w1pt")
            w1cv_sb = loadw(w1cv, [COUT, D1], BF, "w1cv")
            w1gl_sb = loadw(w1gl, [COUT, D1], BF, "w1gl")
            w2_sb = loadw(w2p, [D1, D2], BF, "w2")
            w3a_sb = loadw(w3ap, [D2, 128], BF, "w3a")
            w3b_sb = loadw(w3bp, [D2, 128], BF, "w3b")
            woa_sb = loadw(woap, [128, DO], BF, "woa")
            wob_sb = loadw(wobp, [128, DO], BF, "wob")
            gb1_sb = loadw(gb1p, [D1, 2], F32, "gb1")
            gb2_sb = loadw(gb2p, [D2, 2], F32, "gb2")
            gb3a_sb = loadw(gb3ap, [128, 2], F32, "gb3a")
            gb3b_sb = loadw(gb3bp, [128, 2], F32, "gb3b")
            bout_sb = loadw(boutp, [128, 8, DO], F32, "bout")

            # big SBUF slots (shared lifetimes via tags)
            g_sb = sb.tile([128, P // 128, ROW], BF, tag="slot_g")      # gather
            conv_sb = sb.tile([COUT, P], BF, tag="slot_conv4")
            yraw = sb.tile([128, NCH, 512], F32, tag="slot_yraw")
            h1 = sb.tile([D1, P], BF, tag="slot_h13a")
            out_sb = sb.tile([128, P // 128, DO], F32, tag="outsb")

            imcol_probe = []
            # ---- gather ---------------------------------------------------
            for k in range(NBLK):
                nc.gpsimd.dma_gather(
                    out_ap=g_sb[:, k * (BLK // 128):(k + 1) * (BLK // 128), :],
                    in_ap=srcs[k][:],
                    idxs_ap=idx_sb[:, k, :],
                    num_idxs=BLK,
                    num_idxs_reg=BLK,
                    elem_size=ROW,
                    queue_num=0,
                )

            if _stage == "gather":
                nc.vector.tensor_copy(out=out_sb[:, 0:4, :].bitcast(BF),
                                      in_=g_sb[:, 0, 0:80])
                nc.sync.dma_start(out=out[:], in_=out_sb[:])
            # ---- transpose -> im2col -> conv -> L1-partial ---------------
            ptb = None
            for ch in range(NCH if _stage != "gather" else 0):
                if ch % 4 == 0:
                    ptb = imp.tile([6, 2048], BF, tag="ptb")
                    nc.sync.dma_start(out=ptb[:],
                                      in_=ptT[:, ch * 512:(ch + 4) * 512])
                tp = ps.tile([K_IM, 512], BF, tag="tp")
                for j in range(4):
                    c = ch * 4 + j
                    nc.tensor.transpose(
                        out=tp[:, j * 128:(j + 1) * 128],
                        in_=g_sb[:, c, 0:K_IM],
                        identity=id_sb[:],
                    )
                im = imp.tile([K_IM, 512], BF, tag="im")
                if ch == 0:
                    imcol_probe.append(im)
                if ch % 2 == 0:
                    nc.vector.tensor_copy(out=im[:], in_=tp[:])
                else:
                    nc.scalar.activation(out=im[:], in_=tp[:], func=AF.Copy)

                if _sub == "tp":
                    continue
                cp = ps.tile([COUT, 512], F32, tag="cp")
                nc.tensor.matmul(out=cp[:], lhsT=wext_sb[:],
                                 rhs=im[:], start=True, stop=True)
                cvs = conv_sb[:, ch * 512:(ch + 1) * 512]
                if ch % 2 == 0:
                    nc.scalar.activation(out=cvs, in_=cp[:], func=AF.Copy)
                else:
                    nc.vector.tensor_copy(out=cvs, in_=cp[:])

                if _sub == "cv":
                    continue
                yp = ps.tile([D1, 512], F32, tag="yp")
                nc.tensor.matmul(out=yp[:],
                                 lhsT=w1pt_sb[:],
                                 rhs=ptb[:, (ch % 4) * 512:(ch % 4 + 1) * 512],
                                 start=True, stop=False)
                nc.tensor.matmul(out=yp[:], lhsT=w1cv_sb[:], rhs=cvs,
                                 start=False, stop=True)
                if ch % 2 == 0:
                    nc.vector.tensor_copy(out=yraw[:, ch, :], in_=yp[:])
                else:
                    nc.scalar.activation(out=yraw[:, ch, :], in_=yp[:], func=AF.Copy)

            if _stage in ("conv", "l1"):
                nc.vector.tensor_copy(out=out_sb[:, 0:4, :].bitcast(BF),
                                      in_=g_sb[:, 0, 0:80])
                if _sub in ("l1", "l1b"):
                    nc.vector.tensor_copy(out=out_sb[:, :, :],
                                          in_=yraw[:, 0:10, 0:128])
                if _sub in ("cv", "l1", "l1b"):
                    nc.vector.tensor_copy(out=out_sb[0:32, 0:4, :].bitcast(BF),
                                          in_=conv_sb[:, 0:80])
                if _sub in ("tp", "cv", "l1", "l1b"):
                    nc.vector.tensor_copy(out=out_sb[0:108, 4:8, :].bitcast(BF),
                                          in_=imcol_probe[0][:, 0:80])
                nc.sync.dma_start(out=out[:], in_=out_sb[:])
            if _stage == "full":
                # ---- glob max -> v_g -----------------------------------------
                glob = sb.tile([COUT, 1], BF, tag="glob")
                nc.vector.tensor_reduce(out=glob[:], in_=conv_sb[:],
                                        axis=mybir.AxisListType.X, op=OP.max)
                nc.vector.tensor_scalar(out=glob[:], in0=glob[:],
                                        scalar1=b3d_sb[:, 0:1], scalar2=None,
                                        op0=OP.add)
                vgp = psy.tile([D1, 1], F32, tag="vg")
                nc.tensor.matmul(out=vgp[:], lhsT=w1gl_sb[:],
                                 rhs=glob[:], start=True, stop=True)
                vg = sb.tile([D1, 1], F32, tag="vgs")
                nc.vector.tensor_copy(out=vg[:], in_=vgp[:])

                # ---- helpers --------------------------------------------------
                def bn_sums(y_t, C, tag):
                    st = sb.tile([C, NCH * 6], F32, tag=tag + "st")
                    for ch in range(NCH):
                        nc.vector.bn_stats(out=st[:, ch * 6:(ch + 1) * 6],
                                           in_=y_t[:, ch, :])
                    ag = sb.tile([C, 2], F32, tag=tag + "ag")
                    nc.vector.bn_aggr(out=ag[:], in_=st[:])
                    s = sb.tile([C, 2], F32, tag=tag + "s")
                    nc.vector.tensor_tensor(out=s[:, 1:2], in0=ag[:, 0:1],
                                            in1=ag[:, 0:1], op=OP.mult)
                    nc.vector.tensor_tensor(out=s[:, 1:2], in0=s[:, 1:2],
                                            in1=ag[:, 1:2], op=OP.add)
                    nc.vector.tensor_scalar_mul(out=s[:, 1:2], in0=s[:, 1:2],
                                                scalar1=float(P))
                    nc.vector.tensor_scalar_mul(out=s[:, 0:1], in0=ag[:, 0:1],
                                                scalar1=float(P))
                    return s

                import os as _os
                _skip_cc = _os.environ.get("K_SKIP_CC", "0") == "1"

                def allreduce(t_sb, shape, tag):
                    if _skip_cc:
                        r = sb.tile(shape, F32, tag=tag + "r")
                        nc.vector.tensor_scalar_mul(out=r[:], in0=t_sb[:], scalar1=8.0)
                        return r
                    di = dramp.tile(shape, F32, tag=tag + "i")
                    do = dramp.tile(shape, F32, tag=tag + "o")
                    nc.gpsimd.dma_start(out=di[:], in_=t_sb[:])
                    nc.gpsimd.collective_compute(
                        "AllReduce", OP.add, replica_groups=[list(range(8))],
                        ins=[di.opt()], outs=[do.opt()])
                    r = sb.tile(shape, F32, tag=tag + "r")
                    nc.sync.dma_start(out=r[:], in_=do[:])
                    return r

                def bn_ab(red, gb, C, tag, vg_local=None):
                    a = sb.tile([C, 1], F32, tag=tag + "a")
                    cb = sb.tile([C, 1], F32, tag=tag + "c")
                    m = sb.tile([C, 1], F32, tag=tag + "m")
                    if vg_local is None:
                        nc.vector.tensor_scalar_mul(out=m[:], in0=red[:, 0:1],
                                                    scalar1=1.0 / N_TOT)
                        nc.vector.tensor_scalar_mul(out=a[:], in0=red[:, 1:2],
                                                    scalar1=1.0 / N_TOT)
                    else:
                        nc.vector.tensor_scalar_mul(out=m[:], in0=red[:, 4:5],
                                                    scalar1=float(P))
                        nc.vector.tensor_tensor(out=m[:], in0=m[:], in1=red[:, 0:1],
                                                op=OP.add)
                        nc.vector.tensor_scalar_mul(out=m[:], in0=m[:],
                                                    scalar1=1.0 / N_TOT)
                        t4 = sb.tile([C, 1], F32, tag=tag + "t4")
                        nc.vector.tensor_scalar_mul(out=a[:], in0=red[:, 2:3],
                                                    scalar1=2.0)
                        nc.vector.tensor_tensor(out=a[:], in0=a[:], in1=red[:, 1:2],
                                                op=OP.add)
                        nc.vector.tensor_scalar_mul(out=t4[:], in0=red[:, 3:4],
                                                    scalar1=float(P))
                        nc.vector.tensor_tensor(out=a[:], in0=a[:], in1=t4[:],
                                                op=OP.add)
                        nc.vector.tensor_scalar_mul(out=a[:], in0=a[:],
                                                    scalar1=1.0 / N_TOT)
                    msq = sb.tile([C, 1], F32, tag=tag + "q")
                    nc.vector.tensor_tensor(out=msq[:], in0=m[:], in1=m[:], op=OP.mult)
                    nc.vector.tensor_tensor(out=a[:], in0=a[:], in1=msq[:],
                                            op=OP.subtract)
                    nc.vector.tensor_scalar_add(out=a[:], in0=a[:], scalar1=EPS)
                    nc.scalar.activation(out=a[:], in_=a[:], func=AF.Sqrt)
                    nc.vector.reciprocal(out=a[:], in_=a[:])
                    nc.vector.tensor_tensor(out=a[:], in0=a[:], in1=gb[:, 0:1],
                                            op=OP.mult)
                    if vg_local is not None:
                        nc.vector.tensor_tensor(out=cb[:], in0=vg_local[:], in1=m[:],
                                                op=OP.subtract)
                        nc.vector.tensor_tensor(out=cb[:], in0=cb[:], in1=a[:],
                                                op=OP.mult)
                    else:
                        nc.vector.tensor_tensor(out=cb[:], in0=m[:], in1=a[:],
                                                op=OP.mult)
                        nc.vector.tensor_scalar_mul(out=cb[:], in0=cb[:], scalar1=-1.0)
                    nc.vector.tensor_tensor(out=cb[:], in0=cb[:], in1=gb[:, 1:2],
                                            op=OP.add)
                    return a, cb

                def apply_relu(y_t, h_ap, a, cb):
                    hf = NCH // 2
                    nc.scalar.activation(out=h_ap[:, 0:P // 2], in_=y_t[:, 0:hf, :],
                                         func=AF.Relu, bias=cb[:], scale=a[:])
                    nc.vector.tensor_scalar(out=h_ap[:, P // 2:P], in0=y_t[:, hf:NCH, :],
                                            scalar1=a[:], scalar2=cb[:],
                                            op0=OP.mult, op1=OP.add)
                    nc.vector.tensor_scalar_max(out=h_ap[:, P // 2:P],
                                                in0=h_ap[:, P // 2:P], scalar1=0.0)

                # ---- BN1 ------------------------------------------------------
                s1 = bn_sums(yraw, D1, "b1")
                pk1 = sb.tile([D1, 5], F32, tag="pk1")
                nc.vector.tensor_copy(out=pk1[:, 0:2], in_=s1[:])
                nc.vector.tensor_tensor(out=pk1[:, 2:3], in0=vg[:], in1=s1[:, 0:1],
                                        op=OP.mult)
                nc.vector.tensor_tensor(out=pk1[:, 3:4], in0=vg[:], in1=vg[:],
                                        op=OP.mult)
                nc.vector.tensor_copy(out=pk1[:, 4:5], in_=vg[:])
                red1 = allreduce(pk1, [D1, 5], "r1")
                a1, c1 = bn_ab(red1, gb1_sb, D1, "x1", vg_local=vg)
                apply_relu(yraw, h1[:], a1, c1)

                # ---- L2 -------------------------------------------------------
                h2 = sb.tile([D2, P], BF, tag="slot_g")
                for ch in range(NCH):
                    yp = ps.tile([D2, 512], F32, tag="yp")
                    nc.tensor.matmul(out=yp[:], lhsT=w2_sb[:],
                                     rhs=h1[:, ch * 512:(ch + 1) * 512],
                                     start=True, stop=True)
                    if ch % 2 == 0:
                        nc.scalar.activation(out=yraw[:, ch, :], in_=yp[:], func=AF.Copy)
                    else:
                        nc.vector.tensor_copy(out=yraw[:, ch, :], in_=yp[:])
                s2 = bn_sums(yraw, D2, "b2")
                red2 = allreduce(s2, [D2, 2], "r2")
                a2, c2 = bn_ab(red2, gb2_sb, D2, "x2")
                apply_relu(yraw, h2[:], a2, c2)

                # ---- L3 (sequential halves, yraw slot reused) ----------------
                h3a = sb.tile([128, P], BF, tag="slot_h13a")
                h3b = sb.tile([128, P], BF, tag="slot_conv4")
                for half, (w_sb, gb_sb, h_t, tg) in enumerate(
                    ((w3a_sb, gb3a_sb, h3a, "3a"), (w3b_sb, gb3b_sb, h3b, "3b"))
                ):
                    for ch in range(NCH):
                        yp = ps.tile([128, 512], F32, tag="yp")
                        nc.tensor.matmul(out=yp[:], lhsT=w_sb[:],
                                         rhs=h2[:, ch * 512:(ch + 1) * 512],
                                         start=True, stop=True)
                        if ch % 2 == 0:
                            nc.scalar.activation(out=yraw[:, ch, :], in_=yp[:],
                                                 func=AF.Copy)
                        else:
                            nc.vector.tensor_copy(out=yraw[:, ch, :], in_=yp[:])
                    s3 = bn_sums(yraw, 128, "b" + tg)
                    red3 = allreduce(s3, [128, 2], "r" + tg)
                    a3, c3 = bn_ab(red3, gb_sb, 128, "x" + tg)
                    apply_relu(yraw, h_t[:], a3, c3)

                # ---- L4 (orientation B) --------------------------------------
                for grp in range(P // 1024):
                    op = psy.tile([128, 8, DO], F32, tag="op")
                    for j in range(8):
                        c = grp * 8 + j
                        nc.tensor.matmul(out=op[:, j, :],
                                         lhsT=h3a[:, c * 128:(c + 1) * 128],
                                         rhs=woa_sb[:], start=True, stop=False)
                        nc.tensor.matmul(out=op[:, j, :],
                                         lhsT=h3b[:, c * 128:(c + 1) * 128],
                                         rhs=wob_sb[:], start=False, stop=True)
                    if grp % 2 == 0:
                        nc.vector.tensor_tensor(out=out_sb[:, grp * 8:(grp + 1) * 8, :],
                                                in0=op[:], in1=bout_sb[:], op=OP.add)
                    else:
                        nc.vector.tensor_tensor(out=out_sb[:, grp * 8:(grp + 1) * 8, :],
                                                in0=op[:], in1=bout_sb[:], op=OP.add)
                nc.sync.dma_start(out=out[:], in_=out_sb[:])

    nc.compile()
    return nc


def _mk_w1pt(w1_):
    wp = np.ascontiguousarray(w1_[:, 0:3].T).astype(BF16)   # [3, 128]
    z = np.zeros((6, D1), BF16)
    z[0:3] = wp
    z[3:6] = wp
    return z


def kernel(x, pt_loc, w3d, b3d, w1, b1, g1, beta1, w2, b2, g2, beta2,
           w3, b3, g3, beta3, w_out, b_out, **_unused):
    x = np.asarray(x, np.float32)
    pt_loc = np.asarray(pt_loc, np.float32)

    x_nbr, lin_s, order = _pack_host(x, pt_loc)
    bases_l, tiles_l = [], []
    for b in range(B):
        bs, tl = _build_idx_tables(lin_s[b])
        bases_l.append(bs)
        tiles_l.append(tl)

    if _prog_cache[0] is None:
        _prog_cache[0] = _build_program()
    nc = _prog_cache[0]

    w1_ = np.asarray(w1, np.float32)
    feed = {
        "ident": np.eye(128, dtype=BF16),
        "wext": np.ascontiguousarray(np.asarray(w3d, np.float32).reshape(COUT, K_IM).T).astype(BF16),
        "b3d": np.asarray(b3d, np.float32).reshape(COUT, 1),
        "w1pt": _mk_w1pt(w1_),
        "w1cv": np.ascontiguousarray(w1_[:, 3:35].T).astype(BF16),
        "w1gl": np.ascontiguousarray(w1_[:, 35:67].T).astype(BF16),
        "w2": np.ascontiguousarray(np.asarray(w2, np.float32).T).astype(BF16),
        "w3a": np.ascontiguousarray(np.asarray(w3, np.float32)[0:128, :].T).astype(BF16),
        "w3b": np.ascontiguousarray(np.asarray(w3, np.float32)[128:256, :].T).astype(BF16),
        "wo_a": np.ascontiguousarray(np.asarray(w_out, np.float32)[:, 0:128].T).astype(BF16),
        "wo_b": np.ascontiguousarray(np.asarray(w_out, np.float32)[:, 128:256].T).astype(BF16),
        "gb1": np.stack([np.asarray(g1, np.float32), np.asarray(beta1, np.float32)], 1),
        "gb2": np.stack([np.asarray(g2, np.float32), np.asarray(beta2, np.float32)], 1),
        "gb3a": np.stack([np.asarray(g3, np.float32)[0:128],
                          np.asarray(beta3, np.float32)[0:128]], 1),
        "gb3b": np.stack([np.asarray(g3, np.float32)[128:256],
                          np.asarray(beta3, np.float32)[128:256]], 1),
        "bout": np.broadcast_to(np.asarray(b_out, np.float32), (128, 8, DO)).copy(),
    }

    in_maps = []
    for b in range(B):
        m = dict(feed)
        for k in range(NBLK):
            base = bases_l[b][k]
            win = np.zeros((32768, ROW), BF16)
            avail = min(32768, GRID ** 3 - base)
            win[:avail] = x_nbr[b, base:base + avail]
            m[f"src{k}"] = win
        m["idx"] = tiles_l[b]
        srt = np.take_along_axis(pt_loc[b], order[b][:, None], axis=0)
        sT = srt.T                                   # [3, P] f32
        hi = np.round(sT * 4.0) / 4.0                # exact in bf16 (< 64, res 0.25)
        lo = (sT - hi).astype(BF16)
        pt3 = np.zeros((6, P), BF16)
        pt3[0:3] = hi.astype(BF16)
        pt3[3:6] = lo
        m["ptT"] = pt3
        in_maps.append(m)

    res = run_bass_kernel_spmd(nc, in_maps, core_ids=list(range(8)))

    out = np.zeros((B, DO, P), np.float32)
    for b in range(B):
        ob = res.results[b]["out"]                       # [128, P//128, 10]
        flat = ob.transpose(1, 0, 2).reshape(P, DO)      # col q = c*128+p
        inv = np.empty(P, np.int64)
        inv[order[b]] = np.arange(P)
        out[b] = flat[inv].T
    return out

